# revision 13
# baseline (speedup 1.0000x reference)
"""Trainium2 Bass kernel for nn_Encoder_55490977464569 (binary-tree GRU encoder).

Strategy
--------
Data-parallel over batch: B=16 -> 2 batch elements per NeuronCore (8 cores),
zero collectives. Each core processes its whole tree (32767 nodes) leaves->root
entirely on-chip: all hidden states live in SBUF (bf16), only `targets` is
streamed in (host pre-transposed to feature-major).

v2 engine-balance design (the kernel is elementwise-throughput bound, not
matmul bound):
  - r/z gates use the Sigmoid LUT directly (one merged [rE|rO|z] sigmoid ACT
    per tile); n uses the Tanh LUT. n-ACTs are paired across 2 tiles.
  - the leaf level's z-gate runs on VectorE as a deg-7 odd polynomial of
    tanh(u/2) (custom DVE op, 8 ALU stages), and the leaf blend
    h = (1-z)*n = (1 - t7(u)) * n * 0.5 is a second 3-stage custom op.
    This removes the whole leaf level from the ScalarE critical path.
  - interior r-gates can likewise move to the DVE poly per-tile (knob) to
    balance ScalarE vs VectorE.
  - the d = cs - n and zd = z*d blend ops run on GpSimd (otherwise idle).

Layout: feature-major [128 features (partitions), node*batch rows (free)].
"""

import sys

if "/opt/trn_rl_repo" not in sys.path:
    sys.path.insert(0, "/opt/trn_rl_repo")
if "/opt/trn_rl_repo/concourse" not in sys.path:
    sys.path.insert(0, "/opt/trn_rl_repo/concourse")

import numpy as np
import ml_dtypes

from concourse import bass, mybir, tile, bacc
from concourse import bass_utils

BF16NP = ml_dtypes.bfloat16
F32 = mybir.dt.float32
BF16 = mybir.dt.bfloat16

N_CORES = 8
DEPTH = 15
HID = 128
IN_DIM = 32
OUT_DIM = 64
BATCH = 16
B_LOCAL = BATCH // N_CORES

T_TILE = 512      # parent rows per tile
H_CHUNK = 1024    # hidden-state chunk width (columns) per SBUF tile

ADD = mybir.AluOpType.add
SUB = mybir.AluOpType.subtract
MULT = mybir.AluOpType.mult
TANH = mybir.ActivationFunctionType.Tanh
SIGM = mybir.ActivationFunctionType.Sigmoid

# ---- tuning knobs -----------------------------------------------------------
CLASSIC_LEAF_MOD = 8   # leaf tile k is "classic" (ScalarE path) iff k % MOD == MOD-1; 0 => all poly
POLY_R_MOD = 0         # interior tile k uses DVE poly r-gate iff POLY_R_MOD and k % POLY_R_MOD == 0
GPS_BLEND = True       # run d/zd of interior blend on GpSimd

# deg-7 odd minimax fits of tanh(x/2); leaf fit on |x|<=5.2, interior on 5.5.
# c[k] is the coefficient of x^(2k+1).
C_LEAFZ = (4.82263014e-01, -2.86510209e-02, 1.10769943e-03, -1.66351474e-05)
C_INTR = (4.78622319e-01, -2.71855380e-02, 9.73869670e-04, -1.33312648e-05)

# ---- custom DVE ops ---------------------------------------------------------
_T7_OP = None
_OM_OP = None


def _register_custom_ops():
    global _T7_OP, _OM_OP
    if _T7_OP is not None:
        return
    from concourse import dve_ops
    from concourse.dve_spec import Spec, Src0, Src1, C0, C1, C2, One, sq, lower
    from concourse.dve_table_gen import dve_ver_for
    from concourse.dve_uop import DveOpSpec

    def make(name, body, reference, rd1):
        spec = Spec(body=body, reference=reference)
        for op in dve_ops.OPS:
            if op.name == name:
                return op
        if name not in dve_ops._SUB_OPCODE_FOR_NAME:
            row = max(dve_ops._SUB_OPCODE_FOR_NAME.values()) + 1
            assert row < 0x20
            dve_ops._SUB_OPCODE_FOR_NAME[name] = row
        ver = dve_ver_for("TRN2")
        s = DveOpSpec(name=name, opcode=dve_ops.get_dve_sub_opcode(name),
                      uops=lower(spec, ver=ver), rd1_en=rd1)
        op = dve_ops.DveOp(name, spec, subdim=False, uops_sha={ver: s.sha(ver)})
        dve_ops.OPS.append(op)
        return op

    p = sq(Src0)
    # monic q(x) = x * (C0*p^3 + C1*p^2 + C2*p + 1), p = x^2.
    # ([P,1]-broadcast Src1 mid-chain wedges the DVE on HW, so the x^1
    # coefficient is normalized to 1 and folded into the consumer op.)
    _T7_OP = make(
        "ANT_T7_MONIC",
        (((p * C0 + C1) * p + C2) * p + One) * Src0,
        lambda in0, in1, s0, s1, imm2:
            (((in0 * in0 * s0 + s1) * in0 * in0 + imm2) * in0 * in0 + 1.0) * in0,
        rd1=False)
    # out = (1 - src0*C1) * src1 * C0
    _OM_OP = make(
        "ANT_ONE_MINUS_AC_B_S",
        (One - Src0 * C1) * Src1 * C0,
        lambda in0, in1, s0, s1, imm2: (1.0 - in0 * s1) * in1 * s0,
        rd1=True)


_register_custom_ops()


def _level_rows(depth, b_local):
    return [2**l * b_local for l in range(depth)]


def _zoff(l, b_local):
    # column offset of level l in xz (heap order: nodes 0..N-1)
    return (2**l - 1) * b_local


def _roff(l, b_local):
    # column offset of level l in xr (levels 0..depth-2, each block 2*R_l wide)
    return (2**l - 1) * 2 * b_local


def _leaf_is_classic(k):
    return CLASSIC_LEAF_MOD and (k % CLASSIC_LEAF_MOD == CLASSIC_LEAF_MOD - 1)


def _tile_is_poly_r(k):
    return POLY_R_MOD and (k % POLY_R_MOD == 0)


def build_program(depth=DEPTH, b_local=B_LOCAL, with_mask=False, with_bias=False,
                  dump_h=False):
    """Build the Bass program (same SPMD program for every core)."""
    nc = bacc.Bacc("TRN2", target_bir_lowering=False, debug=False,
                   num_devices=1)
    R = _level_rows(depth, b_local)
    total_z = sum(R)
    total_r = sum(2 * R[l] for l in range(depth - 1)) if depth > 1 else 0
    # classic paths only when biases force them
    leaf_classic = (lambda k: True) if with_bias else _leaf_is_classic
    poly_r = (lambda k: False) if with_bias else _tile_is_poly_r

    xz_d = nc.dram_tensor("xz", [IN_DIM, total_z], BF16, kind="ExternalInput")
    xr_d = None
    if total_r:
        xr_d = nc.dram_tensor("xr", [IN_DIM, total_r], BF16, kind="ExternalInput")
    # packed x for full-size tiles: 4x 32-row strips -> one [128, 512] DMA
    # feeding 4 concurrently row-tiled K=32 matmuls (tile_position packing).
    pack_levels = [l for l in range(depth - 1) if R[l] >= T_TILE]
    pack_off = {}
    off = 0
    for l in pack_levels:
        pack_off[l] = off
        off += R[l]
    xpack_d = None
    if pack_levels:
        xpack_d = nc.dram_tensor("xpack", [128, off], BF16, kind="ExternalInput")
    leaf_pairs = (R[depth - 1] // T_TILE) // 2 if R[depth - 1] >= 2 * T_TILE else 0
    xleaf_d = None
    if leaf_pairs:
        xleaf_d = nc.dram_tensor("xleaf", [128, leaf_pairs * T_TILE], BF16,
                                 kind="ExternalInput")
    # all bf16 weights in one packed array -> a single startup DMA
    # cols: [w_hr | w_hz | w_hn | wx strips | wl strips | wlc strips | w_ir | w_iz | w_in]
    wcat_d = nc.dram_tensor("wcat", [128, 9 * HID], BF16, kind="ExternalInput")
    w_out_d = nc.dram_tensor("w_out", [HID, 2 * OUT_DIM], F32, kind="ExternalInput")
    out_d = nc.dram_tensor("out", [HID, b_local], F32, kind="ExternalOutput")
    hdump_d = None
    if dump_h:
        hdump_d = nc.dram_tensor("hdump", [HID, total_z], BF16,
                                 kind="ExternalOutput")
    if with_bias:
        # per-partition bias columns: [b_r | b_z | b_n | b_zl/2 | b_nl | b_out]
        bias_d = nc.dram_tensor("biases", [HID, 6], F32, kind="ExternalInput")
    if with_mask:
        mask_d = nc.dram_tensor("mask_bc", [HID, total_z], BF16, kind="ExternalInput")

    leaf = depth - 1

    from contextlib import ExitStack
    with tile.TileContext(nc) as tc, ExitStack() as stack:
        consts = stack.enter_context(tc.tile_pool(name="consts", bufs=1))
        hpool = stack.enter_context(tc.tile_pool(name="hpool", bufs=1))
        xpool = stack.enter_context(tc.tile_pool(name="xpool", bufs=6))
        apool = stack.enter_context(tc.tile_pool(name="apool", bufs=3))
        npool = stack.enter_context(tc.tile_pool(name="npool", bufs=2))
        tpool = stack.enter_context(tc.tile_pool(name="tpool", bufs=3))
        psA = stack.enter_context(tc.tile_pool(name="psA", bufs=2, space="PSUM"))
        psB = stack.enter_context(tc.tile_pool(name="psB", bufs=1, space="PSUM"))
        opool = stack.enter_context(tc.tile_pool(name="opool", bufs=1))

        wcat_sb = consts.tile([128, 9 * HID], BF16, name="wcat_sb", tag="wcat_sb")
        nc.sync.dma_start(out=wcat_sb, in_=wcat_d.ap())
        w_hr = wcat_sb[:, 0 * HID:1 * HID]
        w_hz = wcat_sb[:, 1 * HID:2 * HID]
        w_hn = wcat_sb[:, 2 * HID:3 * HID]
        wx_sb = wcat_sb[:, 3 * HID:4 * HID]
        wl_sb = wcat_sb[:, 4 * HID:5 * HID]
        wlc_sb = wcat_sb[:, 5 * HID:6 * HID]
        w_ir = wcat_sb[0:IN_DIM, 6 * HID:7 * HID]
        w_iz = wcat_sb[0:IN_DIM, 7 * HID:8 * HID]
        w_in = wcat_sb[0:IN_DIM, 8 * HID:9 * HID]
        w_out = consts.tile([HID, 2 * OUT_DIM], F32, name="w_out_sb", tag="w_out_sb")
        nc.sync.dma_start(out=w_out, in_=w_out_d.ap())
        if with_bias:
            bias_sb = consts.tile([HID, 6], F32, name="bias_sb", tag="bias_sb")
            nc.sync.dma_start(out=bias_sb, in_=bias_d.ap())
            b_r, b_z, b_n = bias_sb[:, 0:1], bias_sb[:, 1:2], bias_sb[:, 2:3]
            b_zl, b_nl, b_out = bias_sb[:, 3:4], bias_sb[:, 4:5], bias_sb[:, 5:6]

        # hidden-state tiles: h[l][c] is chunk c of level l (bf16)
        h_tiles = []
        for l in range(depth):
            cw = min(H_CHUNK, R[l])
            n_chunks = (R[l] + cw - 1) // cw
            h_tiles.append([
                hpool.tile([HID, cw], BF16, name=f"h_{l}_{c}", tag=f"h_{l}_{c}")
                for c in range(n_chunks)
            ])

        def hview_of(l, t0, T):
            cw = min(H_CHUNK, R[l])
            cidx, coff = t0 // cw, t0 % cw
            return h_tiles[l][cidx][:, coff:coff + T]

        def mask_mul_inplace(view, lvl, col0, width):
            m_sb = xpool.tile([HID, width], BF16, name="m_sb", tag="m_sb")
            nc.sync.dma_start(
                out=m_sb, in_=mask_d.ap()[:, _zoff(lvl, b_local) + col0:
                                          _zoff(lvl, b_local) + col0 + width])
            nc.vector.tensor_mul(view, view, m_sb)

        def t7(out, in_, c):
            # monic: coefficients normalized by the x^1 coefficient c[0]
            nc.vector._custom_dve(_T7_OP, out=out, in0=in_,
                                  s0=float(c[3] / c[0]), s1=float(c[2] / c[0]),
                                  imm2=float(c[1] / c[0]))

        def om(out, q, b, c0):
            # out = (1 - q*c0) * b * 0.5
            nc.vector._custom_dve(_OM_OP, out=out, in0=q, in1=b,
                                  s0=0.5, s1=float(c0))

        # ---------------- leaf level ----------------
        # psum tile layout per leaf tile: [u | v] (u = xi_z preact, v = xi_n)
        Tl = min(T_TILE, R[leaf])
        n_leaf_tiles = R[leaf] // Tl

        def leaf_tail(ps, Tl, t0, k):
            hv = hview_of(leaf, t0, Tl)
            if leaf_classic(k):
                # tanh form: ps holds [u/2 | v] (halved weights)
                zn = apool.tile([HID, 2 * Tl], BF16, name="zn_leaf", tag="act")
                if with_bias:
                    nc.scalar.activation(zn[:, 0:Tl], ps[:, 0:Tl], TANH, bias=b_zl)
                    nc.scalar.activation(zn[:, Tl:2 * Tl], ps[:, Tl:2 * Tl],
                                         TANH, bias=b_nl)
                else:
                    nc.scalar.activation(zn, ps[:, 0:2 * Tl], TANH)
                zz = tpool.tile([HID, Tl], BF16, name="zz_leaf", tag="cs")
                nc.vector.tensor_scalar(zz, zn[:, 0:Tl], -0.5, 0.5,
                                        op0=MULT, op1=ADD)
                nc.vector.tensor_mul(hv, zz, zn[:, Tl:2 * Tl])
            else:
                # poly form: ps holds [u | v] (plain weights)
                q = tpool.tile([HID, Tl], BF16, name="q_leaf", tag="cs")
                t7(q, ps[:, 0:Tl], C_LEAFZ)
                nl = apool.tile([HID, Tl], BF16, name="n_leaf", tag="act_s")
                nc.scalar.activation(nl, ps[:, Tl:2 * Tl], TANH)
                om(hv, q, nl, C_LEAFZ[0])     # h = (1 - t7(u))*n/2
            if with_mask:
                mask_mul_inplace(hv, leaf, t0, Tl)

        for j in range(leaf_pairs):
            # two leaf tiles (2j, 2j+1) share one [128, T] packed x DMA and
            # four concurrently row-tiled K=32 matmuls.
            xp = xpool.tile([128, Tl], BF16, name="xp_leaf", tag="xp")
            nc.sync.dma_start(out=xp, in_=xleaf_d.ap()[:, j * Tl:(j + 1) * Tl])
            pss = []
            for u in range(2):
                k = 2 * j + u
                wsel = wlc_sb if leaf_classic(k) else wl_sb
                ps = psA.tile([HID, 3 * T_TILE], F32, name="ps_leaf", tag="psA")
                for i in range(2):
                    s = 2 * u + i
                    nc.tensor.matmul(ps[:, i * Tl:(i + 1) * Tl],
                                     wsel[32 * s:32 * (s + 1)],
                                     xp[32 * s:32 * (s + 1)],
                                     start=True, stop=True,
                                     tile_position=(32 * s, 0))
                pss.append(ps)
            for u in range(2):
                leaf_tail(pss[u], Tl, (2 * j + u) * Tl, 2 * j + u)

        for k in range(2 * leaf_pairs, n_leaf_tiles):
            t0 = k * Tl
            xz_sb = xpool.tile([IN_DIM, Tl], BF16, name="xz_sb", tag="xz")
            nc.sync.dma_start(out=xz_sb,
                              in_=xz_d.ap()[:, _zoff(leaf, b_local) + t0:
                                            _zoff(leaf, b_local) + t0 + Tl])
            ps = psA.tile([HID, 3 * T_TILE], F32, name="ps_leaf", tag="psA")
            if leaf_classic(k):
                wz, wn = wlc_sb[0:IN_DIM], wlc_sb[32:32 + IN_DIM]
            else:
                wz, wn = w_iz, w_in
            nc.tensor.matmul(ps[:, 0:Tl], wz, xz_sb, start=True, stop=True)
            nc.tensor.matmul(ps[:, Tl:2 * Tl], wn, xz_sb, start=True, stop=True)
            leaf_tail(ps, Tl, t0, k)

        # ---------------- interior levels ----------------
        for l in range(depth - 2, -1, -1):
            T = min(T_TILE, R[l])
            C_child = min(H_CHUNK, R[l + 1])
            packed = l in pack_off
            nt = R[l] // T

            def stage_a(k, l=l, T=T, C_child=C_child, packed=packed):
                """xi + hr + hz matmuls, merged sigmoid ACT, cs, t2."""
                t0 = k * T
                cw = 2 * T
                cidx, coff = (2 * t0) // C_child, (2 * t0) % C_child
                child = h_tiles[l + 1][cidx][:, coff:coff + cw]
                st = {"child": child, "t0": t0}
                isp = poly_r(k) and packed

                ps = psA.tile([HID, 3 * T_TILE], F32, name="ps_rz", tag="psA")
                st["ps"] = ps
                ps_rr = ps[:, 0:cw]
                # z region must start at a PSUM bank boundary (512 f32 cols):
                # first_mm (start=True) clears has_written for the whole bank,
                # so the z group may not share a bank with the r group.
                z0 = max(cw, 512)
                ps_z = ps[:, z0:z0 + T]
                st["z_contig"] = (z0 == cw)
                if packed:
                    # one [128, T] DMA; 4 concurrently row-tiled K=32 matmuls
                    # (xi_r lo, xi_r hi, xi_z, xi_n)
                    xp = xpool.tile([128, T], BF16, name="xp_sb", tag="xp")
                    nc.sync.dma_start(out=xp,
                                      in_=xpack_d.ap()[:, pack_off[l] + t0:
                                                       pack_off[l] + t0 + T])
                    st["xp"] = xp
                    for s, dst in enumerate((ps[:, 0:T], ps[:, T:2 * T], ps_z)):
                        nc.tensor.matmul(dst, wx_sb[32 * s:32 * (s + 1)],
                                         xp[32 * s:32 * (s + 1)],
                                         start=True, stop=False,
                                         tile_position=(32 * s, 0))
                else:
                    xr_sb = xpool.tile([IN_DIM, cw], BF16, name="xr_sb", tag="xr")
                    nc.sync.dma_start(out=xr_sb,
                                      in_=xr_d.ap()[:, _roff(l, b_local) + 2 * t0:
                                                    _roff(l, b_local) + 2 * t0 + cw])
                    xz_sb = xpool.tile([IN_DIM, T], BF16, name="xz_sb", tag="xz")
                    nc.sync.dma_start(out=xz_sb,
                                      in_=xz_d.ap()[:, _zoff(l, b_local) + t0:
                                                    _zoff(l, b_local) + t0 + T])
                    st["xz_sb"] = xz_sb
                    for i in range((cw + 511) // 512):
                        sl = slice(i * 512, min((i + 1) * 512, cw))
                        nc.tensor.matmul(ps_rr[:, sl], w_ir, xr_sb[:, sl],
                                         start=True, stop=False)
                    nc.tensor.matmul(ps_z, w_iz, xz_sb, start=True, stop=False)

                # rr psum: xi_r + h_child @ W_hr, child-row order
                for i in range((cw + 511) // 512):
                    sl = slice(i * 512, min((i + 1) * 512, cw))
                    nc.tensor.matmul(ps_rr[:, sl], w_hr, child[:, sl],
                                     start=False, stop=True)

                # child sum cs = hl + hr
                cs_sb = tpool.tile([HID, T], BF16, name="cs_sb", tag="cs")
                ch3 = child.rearrange("p (g f) -> p g f", f=4)
                cs3 = cs_sb.rearrange("p (g f) -> p g f", f=2)
                nc.vector.tensor_add(cs3, ch3[:, :, 0:2], ch3[:, :, 2:4])
                st["cs_sb"] = cs_sb
                nc.tensor.matmul(ps_z, w_hz, cs_sb, start=False, stop=True)

                t2_sb = tpool.tile([HID, cw], BF16, name="t2_sb", tag="t2")
                if isp:
                    # z-only sigmoid ACT; r via DVE poly; t2 = r*child
                    az = apool.tile([HID, T], BF16, name="az_sb", tag="act_z")
                    nc.scalar.activation(az, ps_z, SIGM,
                                         **(dict(bias=b_z) if with_bias else {}))
                    st["z_view"] = az[:, 0:T]
                    q = apool.tile([HID, cw], BF16, name="q_sb", tag="act_q")
                    t7(q, ps_rr, C_INTR)                 # q = tanh(a/2)/c0
                    om(t2_sb, q, child, -C_INTR[0])      # t2 = (1+t)*child/2 = r*child
                else:
                    # merged [rE | rO | z] sigmoid ACT
                    arz = apool.tile([HID, cw + T], BF16, name="arz_sb",
                                     tag="act_rz")
                    if with_bias or not st["z_contig"]:
                        bk_r = dict(bias=b_r) if with_bias else {}
                        bk_z = dict(bias=b_z) if with_bias else {}
                        nc.scalar.activation(arz[:, 0:cw], ps_rr, SIGM, **bk_r)
                        nc.scalar.activation(arz[:, cw:cw + T], ps_z, SIGM,
                                             **bk_z)
                    else:
                        nc.scalar.activation(arz, ps[:, 0:cw + T], SIGM)
                    st["z_view"] = arz[:, cw:cw + T]
                    nc.vector.tensor_mul(t2_sb, arz[:, 0:cw], child)
                st["t2_sb"] = t2_sb
                return st

            def stage_hn(k, st, ps_n2, side, l=l, T=T, packed=packed):
                """xi_n + hn matmuls into this tile's half of the paired psum."""
                dst = ps_n2[:, side * T:(side + 1) * T]
                if packed:
                    xp = st["xp"]
                    nc.tensor.matmul(dst, wx_sb[96:128], xp[96:128],
                                     start=True, stop=False,
                                     tile_position=(96, 0))
                else:
                    nc.tensor.matmul(dst, w_in, st["xz_sb"],
                                     start=True, stop=False)
                t23 = st["t2_sb"].rearrange("p (g f) -> p g f", f=4)
                nc.tensor.matmul(dst, w_hn, t23[:, :, 0:2],
                                 start=False, stop=False)
                nc.tensor.matmul(dst, w_hn, t23[:, :, 2:4],
                                 start=False, stop=True)

            def blend(k, st, n_view, l=l, T=T):
                """h = n + z*(cs - n)"""
                t0 = st["t0"]
                cs_sb = st["cs_sb"]
                d_sb = tpool.tile([HID, T], BF16, name="d_sb", tag="d")
                zd_sb = tpool.tile([HID, T], BF16, name="zd_sb", tag="zd")
                eng = nc.gpsimd if (GPS_BLEND and not with_bias) else nc.vector
                eng.tensor_sub(d_sb, cs_sb, n_view)
                eng.tensor_mul(zd_sb, st["z_view"], d_sb)
                hv = hview_of(l, t0, T)
                nc.vector.tensor_add(hv, zd_sb, n_view)
                if with_mask:
                    mask_mul_inplace(hv, l, t0, T)

            # process tiles in pairs sharing one n-psum + one tanh ACT
            for p0 in range(0, nt, 2):
                pair = [p0] if p0 + 1 >= nt else [p0, p0 + 1]
                sts = [stage_a(k) for k in pair]
                ps_n2 = psB.tile([HID, 2 * T_TILE], F32, name="ps_n2", tag="psB")
                for i, k in enumerate(pair):
                    stage_hn(k, sts[i], ps_n2, i)
                n2 = npool.tile([HID, len(pair) * T], BF16, name="n2_sb",
                                tag="act_n")
                if with_bias:
                    for i in range(len(pair)):
                        nc.scalar.activation(n2[:, i * T:(i + 1) * T],
                                             ps_n2[:, i * T:(i + 1) * T],
                                             TANH, bias=b_n)
                else:
                    nc.scalar.activation(n2, ps_n2[:, 0:len(pair) * T], TANH)
                for i, k in enumerate(pair):
                    blend(k, sts[i], n2[:, i * T:(i + 1) * T])

        # ---------------- output head ----------------
        h0f = tpool.tile([HID, b_local], F32, name="h0f", tag="h0f")
        nc.vector.tensor_copy(h0f, h_tiles[0][0])
        ps_out = psB.tile([HID, 2 * T_TILE], F32, name="ps_out", tag="psB")
        nc.tensor.matmul(ps_out[:, 0:b_local], w_out, h0f, start=True, stop=True)
        out_sb = opool.tile([HID, b_local], F32, name="out_sb", tag="out_sb")
        if with_bias:
            nc.scalar.activation(out_sb, ps_out[:, 0:b_local],
                                 mybir.ActivationFunctionType.Identity,
                                 bias=b_out)
        else:
            nc.scalar.copy(out_sb, ps_out[:, 0:b_local])
        nc.sync.dma_start(out=out_d.ap(), in_=out_sb)
        if dump_h:
            for l in range(depth):
                cw = min(H_CHUNK, R[l])
                for c, ht in enumerate(h_tiles[l]):
                    o0 = _zoff(l, b_local) + c * cw
                    nc.sync.dma_start(out=hdump_d.ap()[:, o0:o0 + cw], in_=ht)

    nc.compile()
    return nc


def host_prep(inputs, depth=DEPTH, b_local=B_LOCAL, n_cores=N_CORES,
              with_mask=False, with_bias=False):
    """Build per-core input maps from the full problem inputs."""
    t = np.ascontiguousarray(np.asarray(inputs["targets"], np.float32))
    N = t.shape[0]
    assert N == 2**depth - 1 and t.shape[2] == IN_DIM
    R = _level_rows(depth, b_local)

    # feature-major, bf16: [32, N, B]
    xt = np.ascontiguousarray(t.transpose(2, 0, 1)).astype(BF16NP)

    def plain_t(w):
        return np.ascontiguousarray(np.asarray(w, np.float32).T).astype(BF16NP)

    def half_t(w):
        return np.ascontiguousarray(np.asarray(w, np.float32).T * 0.5).astype(BF16NP)

    w_ir_t = plain_t(inputs["W_ir"])
    w_iz_t = plain_t(inputs["W_iz"])
    w_in_t = plain_t(inputs["W_in"])
    w_izh = half_t(inputs["W_iz"])
    w_hr_t = plain_t(inputs["W_hr"])
    w_hz_t = plain_t(inputs["W_hz"])
    w_hn_t = plain_t(inputs["W_hn"])
    w_out = np.ascontiguousarray(
        np.concatenate([np.asarray(inputs["W_mu"], np.float32),
                        np.asarray(inputs["W_lv"], np.float32)], axis=0).T)

    wcat = np.zeros((128, 9 * HID), BF16NP)
    wcat[:, 0 * HID:1 * HID] = w_hr_t
    wcat[:, 1 * HID:2 * HID] = w_hz_t
    wcat[:, 2 * HID:3 * HID] = w_hn_t
    for i, wsrc in enumerate((w_ir_t, w_ir_t, w_iz_t)):         # wx strips (n strip below)
        wcat[32 * i:32 * (i + 1), 3 * HID:4 * HID] = wsrc
    wcat[96:128, 3 * HID:4 * HID] = w_in_t
    for i, wsrc in enumerate((w_iz_t, w_in_t, w_iz_t, w_in_t)):  # wl strips (poly)
        wcat[32 * i:32 * (i + 1), 4 * HID:5 * HID] = wsrc
    for i, wsrc in enumerate((w_izh, w_in_t, w_izh, w_in_t)):    # wlc strips (classic)
        wcat[32 * i:32 * (i + 1), 5 * HID:6 * HID] = wsrc
    wcat[0:IN_DIM, 6 * HID:7 * HID] = w_ir_t
    wcat[0:IN_DIM, 7 * HID:8 * HID] = w_iz_t
    wcat[0:IN_DIM, 8 * HID:9 * HID] = w_in_t

    shared = dict(wcat=wcat, w_out=w_out)
    if with_bias:
        b = {k: np.asarray(inputs[k], np.float32) for k in
             ("b_ir", "b_hr", "b_iz", "b_hz", "b_in", "b_hn", "b_mu", "b_lv")}
        bias = np.zeros((HID, 6), np.float32)
        bias[:, 0] = b["b_ir"] + b["b_hr"]
        bias[:, 1] = b["b_iz"] + b["b_hz"]
        bias[:, 2] = b["b_in"] + b["b_hn"]
        # leaves: child_sum = s = 0, but b_hz / b_hn still apply in the reference
        bias[:, 3] = 0.5 * (b["b_iz"] + b["b_hz"])
        bias[:, 4] = b["b_in"] + b["b_hn"]
        bias[:128, 5] = np.concatenate([b["b_mu"], b["b_lv"]])
        shared["biases"] = bias

    in_maps = []
    for c in range(n_cores):
        b0 = c * b_local
        xz = np.ascontiguousarray(
            xt[:, :, b0:b0 + b_local].reshape(IN_DIM, N * b_local))
        blocks = []
        for l in range(depth - 1):
            blk = xz[:, _zoff(l, b_local):_zoff(l, b_local) + R[l]]
            rep = np.repeat(blk.reshape(IN_DIM, -1, 1, 2), 2, axis=2)
            blocks.append(rep.reshape(IN_DIM, 2 * R[l]))
        m = dict(shared)
        m["xz"] = xz
        xr = np.concatenate(blocks, axis=1) if blocks else None
        if xr is not None:
            m["xr"] = np.ascontiguousarray(xr)
        # packed [128, T] blocks for tile_position-packed xi matmuls
        pack_levels = [l for l in range(depth - 1) if R[l] >= T_TILE]
        if pack_levels:
            pblocks = []
            for l in pack_levels:
                for k in range(R[l] // T_TILE):
                    t0 = k * T_TILE
                    rblk = xr[:, _roff(l, b_local) + 2 * t0:
                              _roff(l, b_local) + 2 * t0 + 2 * T_TILE]
                    zblk = xz[:, _zoff(l, b_local) + t0:
                              _zoff(l, b_local) + t0 + T_TILE]
                    pblocks.append(np.concatenate(
                        [rblk[:, :T_TILE], rblk[:, T_TILE:], zblk, zblk], axis=0))
            m["xpack"] = np.ascontiguousarray(np.concatenate(pblocks, axis=1))
        leaf = depth - 1
        leaf_pairs = (R[leaf] // T_TILE) // 2 if R[leaf] >= 2 * T_TILE else 0
        if leaf_pairs:
            lblocks = []
            for j in range(leaf_pairs):
                za = xz[:, _zoff(leaf, b_local) + 2 * j * T_TILE:
                        _zoff(leaf, b_local) + (2 * j + 1) * T_TILE]
                zb = xz[:, _zoff(leaf, b_local) + (2 * j + 1) * T_TILE:
                        _zoff(leaf, b_local) + (2 * j + 2) * T_TILE]
                lblocks.append(np.concatenate([za, za, zb, zb], axis=0))
            m["xleaf"] = np.ascontiguousarray(np.concatenate(lblocks, axis=1))
        if with_mask:
            mk = np.asarray(inputs["mask"], np.float32)[:, b0:b0 + b_local]
            m["mask_bc"] = np.ascontiguousarray(
                np.broadcast_to(mk.reshape(1, N * b_local),
                                (HID, N * b_local))).astype(BF16NP)
        in_maps.append(m)
    return in_maps


_PROGRAM_CACHE = {}


def _get_program(with_mask, with_bias):
    key = (with_mask, with_bias)
    if key not in _PROGRAM_CACHE:
        _PROGRAM_CACHE[key] = build_program(with_mask=with_mask,
                                            with_bias=with_bias)
    return _PROGRAM_CACHE[key]


def run_on_device(inputs, trace=False, **trace_kw):
    with_mask = not np.all(np.asarray(inputs["mask"]) == 1.0)
    with_bias = any(
        np.any(np.asarray(inputs[k]) != 0.0)
        for k in ("b_ir", "b_hr", "b_iz", "b_hz", "b_in", "b_hn", "b_mu", "b_lv"))
    nc = _get_program(with_mask, with_bias)
    in_maps = host_prep(inputs, with_mask=with_mask, with_bias=with_bias)
    res = bass_utils.run_bass_kernel_spmd(
        nc, in_maps, core_ids=list(range(N_CORES)), trace=trace, **trace_kw)
    mu = np.zeros((BATCH, OUT_DIM), np.float32)
    lv = np.zeros((BATCH, OUT_DIM), np.float32)
    for c in range(N_CORES):
        o = res.results[c]["out"]  # [128, b_local]
        mu[c * B_LOCAL:(c + 1) * B_LOCAL] = o[:OUT_DIM].T
        lv[c * B_LOCAL:(c + 1) * B_LOCAL] = o[OUT_DIM:].T
    return (mu, lv), res


def kernel(**inputs):
    (mu, lv), _ = run_on_device(inputs)
    return mu, lv
